# revision 39
# baseline (speedup 1.0000x reference)
"""Corr1d cost-volume kernel for Trainium2 (8 NeuronCores), V3.

corr[b, d, h, x] = sum_c fL[b,c,h,x] * fR[b,c,h,x-d]  for x >= d, else 0.
Shapes: fL, fR = (4, 64, 256, 512) fp32; out = (4, 48, 256, 512) fp32.

Sharding: data-parallel over (batch, h-half): core i handles b = i//2,
h rows [128*(i%2), 128*(i%2)+128).

Per-core pipeline (per quad = 4 h rows):
  - 64-wide x-blocks; per h row, 8 banded matmuls [c=64 -> 64 x, 112 win]
    packed two-blocks-per-psum-tile on partition halves; windows live at
    stride 128 in psum -> flat view [128, 16 win, 128] fp32
  - band extraction without mask-multiply or fold: each output col k has
    exactly one valid alias j = k + 48a (a in {0,1,2}); so
      ACT: F <- win[:, :, 48:96]            (alias1 default, fp32->fp16)
      DVE: F <- win[:, :, 0:48]   where mA  (alias0 predicated copy)
      DVE: F[0:16] <- win[:,:,96:112] where mC (alias2 predicated copy)
  - 2 output DMAs per batch (3KB lines) on the vector queue
Host: un-rotates the band with a precomputed numpy gather (free) and
assembles the fp32 output; invalid (x < d) lanes hold garbage on-chip
and are zeroed host-side with np.where.
"""
import numpy as np
from contextlib import ExitStack

import concourse.bass as bass
import concourse.tile as tile
import concourse.bacc as bacc
import concourse.mybir as mybir
from concourse import bass_utils
from concourse.ap import AP

B, C, H, W = 4, 64, 256, 512
D = 48
NCORES = 8
HH = H // 2            # h rows per core
NH = 16                # h rows per load batch
NBATCH = HH // NH      # 8
WIN = 112              # rhs window width per 64-block
WSTRIDE = 128          # psum window stride (112 used + 16 pad)
GB = 64                # x-block width
NBLK = W // GB         # 8 blocks per h row
# window start per block: 64b-47 clipped into [0, W-WIN]
SB = [max(0, min(64 * b - 47, W - WIN)) for b in range(NBLK)]

fp16 = mybir.dt.float16
fp32 = mybir.dt.float32


def _make_masks():
    # alias a of output col k reads win col j = k + 48a, holding the
    # correlation at disparity d_a = x - SB[b] - k - 48a.
    # mA[p, 48w + k] = 1 iff alias0 valid (d_0 in [0, 48))
    # mC[p, 16w + k] = 1 iff alias2 valid (d_2 in [0, 48)), k < 16
    # with p = 64*Hc + u, w = 4*hi + t, b = 2t + Hc, x = 64b + u.
    mA = np.zeros((128, 8 * D), dtype=np.uint8)
    mC = np.zeros((128, 8 * 16), dtype=np.uint8)
    for p in range(128):
        Hc, u = divmod(p, GB)
        for w in range(8):
            t = w % 4
            b = 2 * t + Hc
            x = GB * b + u
            for k in range(D):
                if 0 <= x - SB[b] - k < D:
                    mA[p, D * w + k] = 1.0
            for k in range(16):
                if 0 <= x - SB[b] - k - 96 < D:
                    mC[p, 16 * w + k] = 1.0
    return mA, mC


def _build_nc():
    nc = bacc.Bacc("TRN2", target_bir_lowering=False, debug=False,
                   num_devices=NCORES)
    # host pre-tiles inputs to [ib, half, c, h, x] so every load DMA reads
    # one fully-sequential 512KB HBM block (max HBM efficiency)
    fL_d = nc.dram_tensor("fLc", [NBATCH, 2, C, NH // 2, W], fp16,
                          kind="ExternalInput").ap()
    fR_d = nc.dram_tensor("fRc", [NBATCH, 2, C, NH // 2, W], fp16,
                          kind="ExternalInput").ap()
    mA_d = nc.dram_tensor("mAc", [128, 8 * D], mybir.dt.uint8,
                          kind="ExternalInput").ap()
    mC_d = nc.dram_tensor("mCc", [128, 8 * 16], mybir.dt.uint8,
                          kind="ExternalInput").ap()
    # per load-batch: [p, (hpb, pr, hi, t, k)] -> 6KB DMA lines
    dump_d = nc.dram_tensor("dump", [NBATCH, 128, NH * 4 * D], fp16,
                            kind="ExternalOutput").ap()

    with tile.TileContext(nc) as tc, ExitStack() as ctx:
        const_pool = ctx.enter_context(tc.tile_pool(name="const", bufs=1))
        in_pool = ctx.enter_context(tc.tile_pool(name="inp", bufs=4))
        f_pool = ctx.enter_context(tc.tile_pool(name="fold", bufs=4))
        mm_psum = ctx.enter_context(tc.tile_pool(name="mmps", bufs=4, space="PSUM"))

        mA_t = const_pool.tile([128, 8 * D], mybir.dt.uint8)
        mC_t = const_pool.tile([128, 8 * 16], mybir.dt.uint8)
        nc.scalar.dma_start(mA_t[:], mA_d)
        nc.scalar.dma_start(mC_t[:], mC_d)
        mA8_v = mA_t[:].rearrange("p (w k) -> p w k", w=8)
        mC8_v = mC_t[:].rearrange("p (w k) -> p w k", w=8)

        NHH = NH // 2
        tail_F = {}
        for ib in range(NBATCH):
            h0 = ib * NH
            # h rows h0..h0+7 -> partitions 0:64, h0+8..h0+15 -> 64:128
            fl = in_pool.tile([128, NHH * W], fp16, tag="fl")
            fr = in_pool.tile([128, NHH * W], fp16, tag="fr")
            for half in range(2):
                nc.sync.dma_start(
                    fl[64 * half : 64 * half + 64, :]
                    .rearrange("c (h x) -> c h x", h=NHH),
                    fL_d[ib, half],
                )
                nc.gpsimd.dma_start(
                    fr[64 * half : 64 * half + 64, :]
                    .rearrange("c (h x) -> c h x", h=NHH),
                    fR_d[ib, half],
                )

            F = f_pool.tile([128, NH * 4 * D], fp16)
            for hp in range(NH // 2):
                # unit = one h-row pair hp: q' = hi in {0,1}; psum window
                # w = 4*hi + t at cols [128w, 128w+112), 2 banks per unit
                ps = mm_psum.tile([128, 1024], fp32)
                for t in range(4):
                    for Hc in range(2):
                        b = 2 * t + Hc
                        for hi in range(2):
                            w = 4 * hi + t
                            nc.tensor.matmul(
                                ps[64 * Hc : 64 * Hc + 64,
                                   WSTRIDE * w : WSTRIDE * w + WIN],
                                fl[64 * hi : 64 * hi + 64,
                                   W * hp + GB * b : W * hp + GB * b + GB],
                                fr[64 * hi : 64 * hi + 64,
                                   W * hp + SB[b] : W * hp + SB[b] + WIN],
                                start=True,
                                stop=True,
                            )
                v = ps[:].rearrange("p (w c) -> p w c", w=8)
                Fb = (F[:, 8 * D * hp : 8 * D * (hp + 1)]
                      .rearrange("p (w k) -> p w k", w=8))
                # alias1 default (ACT), then predicated alias0/alias2 (DVE)
                nc.scalar.copy(Fb, v[:, :, D : 2 * D])
                nc.vector.copy_predicated(Fb, mA8_v, v[:, :, 0:D])
                nc.vector.copy_predicated(
                    Fb[:, :, 0:16], mC8_v, v[:, :, 2 * D : WIN]
                )
                # one 6KB-line dump per batch on the scalar queue; the
                # last two batches' dumps are split in halves, with three
                # of the four chunks issued post-loop on the (by then
                # idle) input queues so they skip the scalar backlog
                step = 4 if ib >= NBATCH - 2 else 8
                if hp % step == step - 1:
                    g = hp // step
                    c0 = 8 * D * step * g
                    c1 = 8 * D * step * (g + 1)
                    if ib < NBATCH - 2 or (ib == NBATCH - 1 and g == 0):
                        nc.scalar.dma_start(dump_d[ib][:, c0:c1], F[:, c0:c1])
            if ib >= NBATCH - 2:
                tail_F[ib] = F

        c0 = 32 * D
        F6, F7 = tail_F[NBATCH - 2], tail_F[NBATCH - 1]
        nc.gpsimd.dma_start(dump_d[NBATCH - 2][:, :c0], F6[:, :c0])
        nc.sync.dma_start(dump_d[NBATCH - 2][:, c0:], F6[:, c0:])
        nc.sync.dma_start(dump_d[NBATCH - 1][:, c0:], F7[:, c0:])

    nc.compile()
    return nc


_NC_CACHE = None


def _get_nc():
    global _NC_CACHE
    if _NC_CACHE is None:
        _NC_CACHE = _build_nc()
    return _NC_CACHE


def _tile_input(a):
    # [C, HH, W] -> [ib, half, c, h, x] so each (ib, half) is one
    # contiguous 512KB block
    a = a.astype(np.float16).reshape(C, NBATCH, 2, NH // 2, W)
    return np.ascontiguousarray(a.transpose(1, 2, 0, 3, 4))


def make_in_maps(fL, fR):
    mA, mC = _make_masks()
    in_maps = []
    for core in range(NCORES):
        b, half = divmod(core, 2)
        sl = np.s_[b, :, half * HH : half * HH + HH, :]
        in_maps.append({
            "fLc": _tile_input(fL[sl]),
            "fRc": _tile_input(fR[sl]),
            "mAc": mA,
            "mCc": mC,
        })
    return in_maps


_GATHER_CACHE = None


def _gather_tables():
    # out[d, h, x] = dump[ib, hp, P[x], 192*hi + CB[d, x]] if VALID[d, x]
    global _GATHER_CACHE
    if _GATHER_CACHE is None:
        xs = np.arange(W)
        ds = np.arange(D)
        bx = xs // GB
        ux = xs % GB
        Px = 64 * (bx % 2) + ux                          # [W]
        jabs = xs[None, :] - np.asarray(SB)[bx][None, :] - ds[:, None]  # [D, W]
        valid = (jabs >= 0) & (jabs < WIN)
        tb = bx // 2
        cb = D * tb[None, :] + np.where(valid, jabs, 0) % D             # [D, W]
        FI = Px[None, :] * (4 * D) + cb                  # [D, W] into [p, 192]
        _GATHER_CACHE = (FI.astype(np.int64), valid)
    return _GATHER_CACHE


def kernel(fL: np.ndarray, fR: np.ndarray) -> np.ndarray:
    fL = np.asarray(fL, dtype=np.float32)
    fR = np.asarray(fR, dtype=np.float32)
    nc = _get_nc()
    in_maps = make_in_maps(fL, fR)
    res = bass_utils.run_bass_kernel_spmd(nc, in_maps, core_ids=list(range(NCORES)))

    FI, valid = _gather_tables()
    out = np.empty((B, D, H, W), dtype=np.float32)
    for core in range(NCORES):
        b, half = divmod(core, 2)
        dump = res.results[core]["dump"]                 # [NBATCH, 128, NH*192]
        # [ib, p, hpb, pr, hi, c] -> [h = (ib, hi, hpb, pr), p*192 + c]
        arr = (dump.reshape(NBATCH, 128, NH // 4, 2, 2, 4 * D)
               .transpose(0, 4, 2, 3, 1, 5)
               .reshape(HH, 128 * 4 * D))
        g = arr[:, FI.reshape(-1)].reshape(HH, D, W).astype(np.float32)
        g = np.where(valid[None, :, :], g, 0.0)
        out[b, :, half * HH : half * HH + HH, :] = g.transpose(1, 0, 2)
    return out
